# revision 17
# baseline (speedup 1.0000x reference)
"""Decoder layer (self-attn + cross-attn + FFN, 3 post-LNs) on 8 TRN2 cores.

Sharding: core c -> (batch b = c//4, query block q = c%4 of 512 tokens).
Uniform SPMD program; causality via per-core data (permuted key order puts the
own block first, so the diagonal is always key-tiles [0, S/128) and
fully-masked keys get an exp bias of -30 from a per-core bias column).
All matmuls run as float32r (fp32 storage, ~tf32 precision, bf16 speed).
"""
import sys, os, uuid, hashlib, types, glob
sys.path.insert(0, '/opt/trn_rl_repo')
import numpy as np

import concourse.bass as bass
import concourse.bacc as bacc
import concourse.mybir as mybir
import concourse.tile as tile

F32 = mybir.dt.float32
F32R = mybir.dt.float32r
AF = mybir.ActivationFunctionType
ALU = mybir.AluOpType

NEFF_CACHE = os.environ.get("MY_NEFF_CACHE", "/root/.my_neff_cache")


# ---------------------------------------------------------------- program ---
def build_program(D=1024, H=16, FF=4096, S=512, E=2048, n_cores=8):
    HD = 64
    assert D == H * HD
    KD = D // 128          # contraction tiles over model dim
    NP = H // 2            # head pairs (=feature 128-tiles of q/k)
    TT = S // 128          # token tiles of own block
    KT = E // 128          # key tiles
    FT = FF // 128
    DIAG = S // 128        # first DIAG key tiles are the causal diagonal
    VW = H * 128           # V_ext width
    CH = min(512, D)       # free-dim chunk over model features
    NOF = D // CH
    KC = min(512, E)       # key chunk
    TC = min(512, E)       # token chunk for projections
    NTC = E // TC

    nc = bacc.Bacc("TRN2", target_bir_lowering=False, debug=False,
                   num_devices=n_cores)

    def din(name, shape, dt=F32R):
        return nc.dram_tensor(name, shape, dt, kind="ExternalInput")

    yT_d = din("yT", [D, E])
    yblk_d = din("yblk", [S, D], F32)
    xT_d = din("xT", [D, E])
    kbias_d = din("kbias", [128, KT], F32)
    ident_d = din("ident", [128, 128])
    ones128_d = din("ones128", [1, 128])
    ozsb_d = din("ozsb", [128, H * 64])
    wq_d, wk_d, wv_d = din("wq", [D, D]), din("wk", [D, D]), din("wv", [D, D])
    wsa_d = din("wsa", [D, D])
    wq2_d, wk2_d, wv2_d = din("wq2", [D, D]), din("wk2", [D, D]), din("wv2", [D, D])
    wca_d = din("wca", [D, D])
    wff1_d, wff2_d = din("wff1", [D, FF]), din("wff2", [FF, D])
    bqT_d, bkT_d = din("bqT", [128, NP], F32), din("bkT", [128, NP], F32)
    bq2T_d, bk2T_d = din("bq2T", [128, NP], F32), din("bk2T", [128, NP], F32)
    bff1T_d = din("bff1T", [128, FT], F32)
    bv_d, bv2_d = din("bv_r", [1, D]), din("bv2_r", [1, D])
    bsa_d, bca_d, bff2_d = din("bsa_r", [1, D]), din("bca_r", [1, D]), din("bff2_r", [1, D])
    gb_d = {n: din(n, [1, D], F32) for n in ("g1", "b1", "g2", "b2", "g3", "b3")}

    out_d = nc.dram_tensor("out_blk", [S, D], F32, kind="ExternalOutput")

    kfm_s = nc.dram_tensor("kfm_s", [D, E], F32R)
    vext_s = nc.dram_tensor("vext_s", [128, (E // 128) * VW], F32R)
    k2fm_s = nc.dram_tensor("k2fm_s", [D, E], F32R)
    v2ext_s = nc.dram_tensor("v2ext_s", [128, (E // 128) * VW], F32R)

    with tile.TileContext(nc) as tc:
        cpool = tc.alloc_tile_pool(name="const", bufs=1)
        ident = cpool.tile([128, 128], F32R, tag="ident")
        ones128 = cpool.tile([1, 128], F32R, tag="ones128")
        ozsb = cpool.tile([128, H * 64], F32R, tag="ozsb")
        kbias = cpool.tile([128, KT], F32, tag="kbias")
        eps_t = cpool.tile([128, 1], F32, tag="eps")
        nc.vector.memset(eps_t[:], 1e-9)
        for t, d in [(ident, ident_d), (ones128, ones128_d), (ozsb, ozsb_d),
                     (kbias, kbias_d)]:
            nc.sync.dma_start(t[:], d[:])
        bqT = cpool.tile([128, NP], F32, tag="bqT")
        bkT = cpool.tile([128, NP], F32, tag="bkT")
        bq2T = cpool.tile([128, NP], F32, tag="bq2T")
        bk2T = cpool.tile([128, NP], F32, tag="bk2T")
        bff1T = cpool.tile([128, FT], F32, tag="bff1T")
        for t, d in [(bqT, bqT_d), (bkT, bkT_d), (bq2T, bq2T_d),
                     (bk2T, bk2T_d), (bff1T, bff1T_d)]:
            nc.sync.dma_start(t[:], d[:])
        brow = {}
        for nm, d in [("bv", bv_d), ("bv2", bv2_d), ("bsa", bsa_d),
                      ("bca", bca_d), ("bff2", bff2_d)]:
            brow[nm] = cpool.tile([1, D], F32R, tag="br_" + nm)
            nc.sync.dma_start(brow[nm][:], d[:])
        gb = {}
        for nm, d in gb_d.items():
            row = cpool.tile([1, D], F32, tag="rowld")
            nc.sync.dma_start(row[:], d[:])
            gb[nm] = cpool.tile([128, D], F32, tag="bb_" + nm)
            nc.gpsimd.partition_broadcast(gb[nm][:], row[:], channels=128)

        p_blk = tc.alloc_tile_pool(name="p_blk", bufs=1)
        yblk = [p_blk.tile([128, D], F32, tag=f"yblk{t}") for t in range(TT)]
        for t in range(TT):
            nc.sync.dma_start(yblk[t][:], yblk_d[t * 128:(t + 1) * 128, :])

        p_att = tc.alloc_tile_pool(name="p_att", bufs=1)
        qfm = [p_att.tile([128, S], F32R, tag=f"qfm{p}") for p in range(NP)]
        avfm = [p_att.tile([128, S], F32R, tag=f"avfm{p}") for p in range(NP)]

        # ------------- chunked K/V/Q projections (src streamed) ------------
        def kvq_proj(*a, **kw):
            for _ in kvq_gen(*a, **kw):
                pass

        def kvq_gen(srcT_d, wq_, wk_, wv_, bq_, bk_, bv_, q_dst, kfm_dst,
                    vext_dst, with_q, src_bufs=2, ps_bufs=2):
            with (
                tc.tile_pool(name="pj_src", bufs=src_bufs) as srp,
                tc.tile_pool(name="pj_wk", bufs=1) as wkp,
                tc.tile_pool(name="pj_wv", bufs=2) as wvp,
                tc.tile_pool(name="pj_ev", bufs=2) as evp,
                tc.tile_pool(name="pj_ps", bufs=ps_bufs, space="PSUM") as pp,
            ):
                # ones/zeros filler into vext scratch
                for tb in range(KT):
                    nc.gpsimd.dma_start(
                        vext_dst[:, tb * VW:(tb + 1) * VW]
                        .rearrange("p (h c) -> p h c", c=128)[:, :, 64:128],
                        ozsb[:].rearrange("p (h c) -> p h c", c=64))
                wk_t = []
                for k in range(KD):
                    t = wkp.tile([128, D], F32R, tag=f"wk{k}")
                    nc.sync.dma_start(t[:], wk_[k * 128:(k + 1) * 128, :])
                    wk_t.append(t)
                if with_q:
                    wq_t = []
                    for k in range(KD):
                        t = wkp.tile([128, D], F32R, tag=f"wq{k}")
                        nc.sync.dma_start(t[:], wq_[k * 128:(k + 1) * 128, :])
                        wq_t.append(t)
                for tch in range(NTC):
                    src = []
                    for k in range(KD):
                        t = srp.tile([128, TC], F32R, tag=f"src{k}")
                        nc.sync.dma_start(
                            t[:], srcT_d[k * 128:(k + 1) * 128,
                                         tch * TC:(tch + 1) * TC])
                        src.append(t)
                    # K projection, feature-major out
                    for p in range(NP):
                        ps = pp.tile([128, TC], F32, tag="psk")
                        for k in range(KD):
                            nc.tensor.matmul(
                                ps[:], wk_t[k][:, p * 128:(p + 1) * 128],
                                src[k][:], start=(k == 0), stop=(k == KD - 1))
                        ev = evp.tile([128, TC], F32R, tag="kev")
                        nc.scalar.activation(ev[:], ps[:], AF.Identity,
                                             bias=bk_[:, p:p + 1])
                        nc.gpsimd.dma_start(
                            kfm_dst[p * 128:(p + 1) * 128,
                                    tch * TC:(tch + 1) * TC], ev[:])
                    # Q projection (own block = chunk 0, cols 0:S)
                    if with_q and tch == 0:
                        for p in range(NP):
                            ps = pp.tile([128, S], F32, tag="psq")
                            for k in range(KD):
                                nc.tensor.matmul(
                                    ps[:], wq_t[k][:, p * 128:(p + 1) * 128],
                                    src[k][:, 0:S], start=(k == 0),
                                    stop=(k == KD - 1))
                            nc.vector.tensor_scalar_add(q_dst[p][:], ps[:],
                                                        bq_[:, p:p + 1])
                    # V projection, token-major out into vext scratch
                    wv_t = []
                    for k in range(KD):
                        t = wvp.tile([128, D], F32R, tag=f"wv{k % 2}")
                        nc.sync.dma_start(t[:], wv_[k * 128:(k + 1) * 128, :])
                        wv_t.append(t)
                    for tt_ in range(TC // 128):
                        gtok = tch * TC + tt_ * 128
                        for vf in range(NOF):
                            psv = pp.tile([128, CH], F32, tag="psv")
                            for k in range(KD):
                                nc.tensor.matmul(
                                    psv[:], src[k][:, tt_ * 128:(tt_ + 1) * 128],
                                    wv_t[k][:, vf * CH:(vf + 1) * CH],
                                    start=(k == 0), stop=False)
                            nc.tensor.matmul(psv[:], ones128[:],
                                             bvrow[:, vf * CH:(vf + 1) * CH],
                                             start=False, stop=True)
                            ev = evp.tile([128, CH], F32R, tag="vev")
                            nc.scalar.copy(ev[:], psv[:])
                            nhd = CH // 64
                            kt_ = gtok // 128
                            nc.gpsimd.dma_start(
                                vext_dst[:, kt_ * VW:(kt_ + 1) * VW]
                                .rearrange("p (h c) -> p h c", c=128)
                                [:, vf * nhd:(vf + 1) * nhd, 0:64],
                                ev[:].rearrange("p (h c) -> p h c", c=64))
                    yield

        kvq_proj(yT_d, wq_d, wk_d, wv_d, bqT, bkT, bv_d, qfm, kfm_s,
                 vext_s, with_q=True)

        # ------------------------- attention -------------------------------
        def attention(*a, **kw):
            for _ in att_gen(*a, **kw):
                pass

        def att_gen(q_tiles, kfm_src, vext_src, out_tiles, causal,
                    ss_bufs=2, pav_bufs=2):
            with (
                tc.tile_pool(name="at_sb", bufs=3) as sp,
                tc.tile_pool(name="at_ps", bufs=ss_bufs, space="PSUM") as pp,
                tc.tile_pool(name="at_pav", bufs=pav_bufs, space="PSUM") as pav,
                tc.tile_pool(name="at_n", bufs=2) as npool,
            ):
                for p in range(NP):
                    psA = pav.tile([128, S], F32, tag="psavA")
                    psB = pav.tile([128, S], F32, tag="psavB")
                    for ktc in range(E // KC):
                        kl = sp.tile([128, KC], F32R, tag="kl")
                        nc.sync.dma_start(
                            kl[:], kfm_src[p * 128:(p + 1) * 128,
                                           ktc * KC:(ktc + 1) * KC])
                        for kj in range(KC // 128):
                            kt = ktc * (KC // 128) + kj
                            vl = sp.tile([128, 256], F32R, tag="vl")
                            nc.sync.dma_start(
                                vl[:], vext_src[kt * 128:(kt + 1) * 128,
                                                p * 256:(p + 1) * 256])
                            ssA = pp.tile([128, S], F32, tag="ssA")
                            ssB = pp.tile([128, S], F32, tag="ssB")
                            nc.tensor.matmul(
                                ssA[:], kl[0:64, kj * 128:(kj + 1) * 128],
                                q_tiles[p][0:64, :], start=True, stop=True,
                                tile_position=(0, 0))
                            nc.tensor.matmul(
                                ssB[:], kl[64:128, kj * 128:(kj + 1) * 128],
                                q_tiles[p][64:128, :], start=True, stop=True,
                                tile_position=(64, 0))
                            prA = sp.tile([128, S], F32R, tag="prA")
                            prB = sp.tile([128, S], F32R, tag="prB")
                            bias_ap = kbias[:, kt:kt + 1] if causal else 0.0
                            nc.scalar.activation(prA[:], ssA[:], AF.Exp,
                                                 bias=bias_ap, scale=0.125)
                            nc.scalar.activation(prB[:], ssB[:], AF.Exp,
                                                 bias=bias_ap, scale=0.125)
                            if causal and kt < DIAG:
                                for pr in (prA, prB):
                                    nc.gpsimd.affine_select(
                                        out=pr[:], in_=pr[:],
                                        compare_op=ALU.is_ge, fill=0.0,
                                        base=-kt * 128, channel_multiplier=-1,
                                        pattern=[[1, S]])
                            nc.tensor.matmul(psA[:], vl[:, 0:128], prA[:],
                                             start=(kt == 0),
                                             stop=(kt == KT - 1))
                            nc.tensor.matmul(psB[:], vl[:, 128:256], prB[:],
                                             start=(kt == 0),
                                             stop=(kt == KT - 1))
                    recA = npool.tile([1, S], F32, tag="recA")
                    recB = npool.tile([1, S], F32, tag="recB")
                    nc.vector.reciprocal(recA[:], psA[64:65, :])
                    nc.vector.reciprocal(recB[:], psB[64:65, :])
                    rbA = npool.tile([64, S], F32, tag="rbA")
                    rbB = npool.tile([64, S], F32, tag="rbB")
                    nc.gpsimd.partition_broadcast(rbA[:], recA[:], channels=64)
                    nc.gpsimd.partition_broadcast(rbB[:], recB[:], channels=64)
                    avB = npool.tile([64, S], F32R, tag="avB")
                    nc.vector.tensor_mul(out_tiles[p][0:64, :], psA[0:64, :],
                                         rbA[:])
                    nc.vector.tensor_mul(avB[:], psB[0:64, :], rbB[:])
                    nc.sync.dma_start(out_tiles[p][64:128, :], avB[:])
                    yield
                yield  # hold pools open until the driver drains us last

        # Interleave self-attention (ACT-heavy) with cross K/V projection
        # (PE-heavy, independent) so the PE fills attention's exp-wait gaps.
        g_att = att_gen(qfm, kfm_s, vext_s, avfm, causal=True,
                        ss_bufs=1, pav_bufs=1)
        g_cross = kvq_gen(xT_d, None, wk2_d, wv2_d, None, bk2T, bv2_d, None,
                          k2fm_s, v2ext_s, with_q=False, src_bufs=1, ps_bufs=1)
        att_turns = 1
        next(g_att)             # opens att pools first (released last)
        while True:
            try:
                next(g_cross)
            except StopIteration:
                break
            for _ in range(2):
                if att_turns < NP:      # stop at the terminal hold-yield
                    next(g_att)
                    att_turns += 1
        for _ in g_att:
            pass

        # ------------------ fc + residual + LN (token-major) ---------------
        def fc_ln(act_tiles, w_d_, brow_, resid, g_, b_, out_tiles, nk):
            with (
                tc.tile_pool(name="fc_w", bufs=3) as wp,
                tc.tile_pool(name="fc_ps", bufs=1, space="PSUM") as pp,
                tc.tile_pool(name="fc_sb", bufs=1) as sp,
            ):
                ps = [[pp.tile([128, CH], F32, tag=f"fc{t}_{f}")
                       for f in range(NOF)] for t in range(TT)]
                for k in range(nk):
                    wt = wp.tile([128, D], F32R, tag="w")
                    nc.sync.dma_start(wt[:], w_d_[k * 128:(k + 1) * 128, :])
                    for t in range(TT):
                        for f in range(NOF):
                            nc.tensor.matmul(
                                ps[t][f][:],
                                act_tiles[k][:, t * 128:(t + 1) * 128],
                                wt[:, f * CH:(f + 1) * CH],
                                start=(k == 0), stop=False)
                for t in range(TT):
                    for f in range(NOF):
                        nc.tensor.matmul(ps[t][f][:], ones128[:],
                                         brow_[:, f * CH:(f + 1) * CH],
                                         start=False, stop=True)
                for t in range(TT):
                    r = sp.tile([128, D], F32, tag="r")
                    rs = [sp.tile([128, 1], F32, tag=f"rs{f}")
                          for f in range(NOF)]
                    for f in range(NOF):
                        nc.vector.scalar_tensor_tensor(
                            r[:, f * CH:(f + 1) * CH], ps[t][f][:], 1.0,
                            resid[t][:, f * CH:(f + 1) * CH],
                            op0=ALU.mult, op1=ALU.add, accum_out=rs[f][:])
                    rowsum = sp.tile([128, 1], F32, tag="rowsum")
                    if NOF == 2:
                        nc.vector.tensor_add(rowsum[:], rs[0][:], rs[1][:])
                    else:
                        nc.vector.tensor_copy(rowsum[:], rs[0][:])
                    negmean = sp.tile([128, 1], F32, tag="negmean")
                    nc.scalar.mul(negmean[:], rowsum[:], -1.0 / D)
                    xnl = sp.tile([128, D], F32, tag="xnl")
                    nc.scalar.activation(xnl[:], r[:], AF.Identity,
                                         bias=negmean[:])
                    xsq = sp.tile([128, D], F32, tag="xsq")
                    ssq = sp.tile([128, 1], F32, tag="ssq")
                    nc.scalar.activation(xsq[:], xnl[:], AF.Square,
                                         accum_out=ssq[:])
                    sd = sp.tile([128, 1], F32, tag="sd")
                    nc.scalar.activation(sd[:], ssq[:], AF.Sqrt,
                                         bias=eps_t[:], scale=1.0 / D)
                    rstd = sp.tile([128, 1], F32, tag="rstd")
                    nc.vector.reciprocal(rstd[:], sd[:])
                    tmp = sp.tile([128, D], F32, tag="tmp")
                    nc.vector.scalar_tensor_tensor(
                        tmp[:], xnl[:], rstd[:], g_[:], op0=ALU.mult,
                        op1=ALU.mult)
                    nc.vector.tensor_add(out_tiles[t][:], tmp[:], b_[:])

        p_y1 = tc.alloc_tile_pool(name="p_y1", bufs=1)
        y1 = [p_y1.tile([128, D], F32, tag=f"y1_{t}") for t in range(TT)]
        fc_ln(avfm, wsa_d, brow["bsa"], yblk, gb["g1"], gb["b1"], y1, NP)
        p_blk.release()

        p_yT = tc.alloc_tile_pool(name="p_yT", bufs=1)
        yT12 = [p_yT.tile([128, S], F32R, tag=f"yT12_{k}") for k in range(KD)]

        def transpose_to(src_tiles, dst_tiles):
            with (
                tc.tile_pool(name="tp_ps", bufs=2, space="PSUM") as pp,
                tc.tile_pool(name="tp_sb", bufs=2) as sp,
            ):
                for t in range(TT):
                    srcr = sp.tile([128, D], F32R, tag="srcr")
                    nc.vector.tensor_copy(srcr[:], src_tiles[t][:])
                    for k in range(KD):
                        pst = pp.tile([128, 128], F32R, tag="tp")
                        nc.tensor.transpose(pst[:],
                                            srcr[:, k * 128:(k + 1) * 128],
                                            ident[:])
                        nc.vector.tensor_copy(
                            dst_tiles[k][:, t * 128:(t + 1) * 128], pst[:])

        transpose_to(y1, yT12)

        # ------------------------- cross attention -------------------------
        with (
            tc.tile_pool(name="q2_w", bufs=2) as wp,
            tc.tile_pool(name="q2_ps", bufs=2, space="PSUM") as pp,
        ):
            for p in range(NP):
                psq = pp.tile([128, S], F32, tag="psq2")
                for k in range(KD):
                    wt = wp.tile([128, 128], F32R, tag="wq2")
                    nc.sync.dma_start(
                        wt[:], wq2_d[k * 128:(k + 1) * 128,
                                     p * 128:(p + 1) * 128])
                    nc.tensor.matmul(psq[:], wt[:], yT12[k][:, 0:S],
                                     start=(k == 0), stop=(k == KD - 1))
                nc.vector.tensor_scalar_add(qfm[p][:], psq[:],
                                            bq2T[:, p:p + 1])

        attention(qfm, k2fm_s, v2ext_s, avfm, causal=False)
        p_y2 = tc.alloc_tile_pool(name="p_y2", bufs=1)
        y2 = [p_y2.tile([128, D], F32, tag=f"y2_{t}") for t in range(TT)]
        fc_ln(avfm, wca_d, brow["bca"], y1, gb["g2"], gb["b2"], y2, NP)
        p_y1.release()
        transpose_to(y2, yT12)
        p_att.release()

        # ------------------------------ FFN ---------------------------------
        p_h = tc.alloc_tile_pool(name="p_h", bufs=1)
        hfm = [p_h.tile([128, S], F32R, tag=f"h{f}") for f in range(FT)]
        with (
            tc.tile_pool(name="f1_w", bufs=4) as wp,
            tc.tile_pool(name="f1_ps", bufs=2, space="PSUM") as pp,
        ):
            for f in range(FT):
                psf = pp.tile([128, S], F32, tag="psf")
                for k in range(KD):
                    wt = wp.tile([128, 128], F32R, tag="wff1")
                    nc.sync.dma_start(
                        wt[:], wff1_d[k * 128:(k + 1) * 128,
                                      f * 128:(f + 1) * 128])
                    nc.tensor.matmul(psf[:], wt[:], yT12[k][:, 0:S],
                                     start=(k == 0), stop=(k == KD - 1))
                nc.scalar.activation(hfm[f][:], psf[:], AF.Relu,
                                     bias=bff1T[:, f:f + 1])
        p_yT.release()

        out_f = [p_h.tile([128, D], F32, tag=f"out{t}") for t in range(TT)]
        fc_ln(hfm, wff2_d, brow["bff2"], y2, gb["g3"], gb["b3"], out_f, FT)
        for t in range(TT):
            nc.sync.dma_start(out_d[t * 128:(t + 1) * 128, :], out_f[t][:])
        p_y2.release()
        p_h.release()
        cpool.release()

    nc.compile()
    return nc


# ---------------------------------------------------------------- hosting ---
def make_inputs_for_core(full, b, o, D=1024, H=16, FF=4096, S=512, E=2048):
    HD = D // H
    KT = E // 128
    y = np.asarray(full["y"][b], dtype=np.float32)      # [E, D]
    x = np.asarray(full["x"][b], dtype=np.float32)
    perm = np.concatenate([np.arange(o, o + S), np.arange(0, o),
                           np.arange(o + S, E)])
    yT = np.ascontiguousarray(y.T[:, perm])
    xT = np.ascontiguousarray(x.T)
    kbias = np.zeros((128, KT), np.float32)
    idx = np.arange(E).reshape(KT, 128).T               # [128, KT]
    kbias[idx >= S + o] = -30.0

    qkv_w = np.asarray(full["qkv_w"], np.float32).reshape(D, H, 3 * HD)
    wq = np.ascontiguousarray(qkv_w[:, :, 0:HD].reshape(D, D))
    wk = np.ascontiguousarray(qkv_w[:, :, HD:2 * HD].reshape(D, D))
    wv = np.ascontiguousarray(qkv_w[:, :, 2 * HD:].reshape(D, D))
    qkv_b = np.asarray(full["qkv_b"], np.float32).reshape(H, 3 * HD)
    bq = qkv_b[:, 0:HD].reshape(D)
    bk = qkv_b[:, HD:2 * HD].reshape(D)
    bv = qkv_b[:, 2 * HD:].reshape(D)
    kv_w = np.asarray(full["kv_w"], np.float32).reshape(D, H, 2 * HD)
    wk2 = np.ascontiguousarray(kv_w[:, :, 0:HD].reshape(D, D))
    wv2 = np.ascontiguousarray(kv_w[:, :, HD:].reshape(D, D))
    kv_b = np.asarray(full["kv_b"], np.float32).reshape(H, 2 * HD)
    bk2 = kv_b[:, 0:HD].reshape(D)
    bv2 = kv_b[:, HD:].reshape(D)

    def colT(v):   # [D] -> [128, D//128] (partition-major per 128-tile)
        return np.ascontiguousarray(v.reshape(-1, 128).T.astype(np.float32))

    ozsb = np.zeros((128, H * 64), np.float32)
    for h in range(H):
        ozsb[:, h * 64:h * 64 + 32] = 1.0

    return {
        "yT": yT, "yblk": np.ascontiguousarray(y[o:o + S]), "xT": xT,
        "kbias": kbias, "ident": np.eye(128, dtype=np.float32),
        "ones128": np.ones((1, 128), np.float32), "ozsb": ozsb,
        "wq": wq, "wk": wk, "wv": wv,
        "wsa": np.asarray(full["sa_fc_w"], np.float32),
        "wq2": np.asarray(full["q_w"], np.float32), "wk2": wk2, "wv2": wv2,
        "wca": np.asarray(full["ca_fc_w"], np.float32),
        "wff1": np.asarray(full["ff1_w"], np.float32),
        "wff2": np.asarray(full["ff2_w"], np.float32),
        "bqT": colT(bq), "bkT": colT(bk),
        "bq2T": colT(np.asarray(full["q_b"], np.float32)), "bk2T": colT(bk2),
        "bff1T": colT(np.asarray(full["ff1_b"], np.float32)),
        "bv_r": bv.reshape(1, D), "bv2_r": bv2.reshape(1, D),
        "bsa_r": np.asarray(full["sa_fc_b"], np.float32).reshape(1, D),
        "bca_r": np.asarray(full["ca_fc_b"], np.float32).reshape(1, D),
        "bff2_r": np.asarray(full["ff2_b"], np.float32).reshape(1, D),
        "g1": np.asarray(full["g1"], np.float32).reshape(1, D),
        "b1": np.asarray(full["b1"], np.float32).reshape(1, D),
        "g2": np.asarray(full["g2"], np.float32).reshape(1, D),
        "b2": np.asarray(full["b2"], np.float32).reshape(1, D),
        "g3": np.asarray(full["g3"], np.float32).reshape(1, D),
        "b3": np.asarray(full["b3"], np.float32).reshape(1, D),
    }


# ------------------------------------------------------------------ runner --
def _install_neff_cache():
    from concourse import bass2jax
    if getattr(bass2jax, "_my_cache_installed", False):
        return
    os.makedirs(NEFF_CACHE, exist_ok=True)
    orig = bass2jax.compile_bir_kernel

    def cached(ant_bir_str, compile_dir_path, neff_name=None, **kw):
        key_bytes = ant_bir_str.encode() if isinstance(ant_bir_str, str) else ant_bir_str
        cpath = os.path.join(NEFF_CACHE,
                             hashlib.sha256(key_bytes).hexdigest() + ".neff")
        if os.path.exists(cpath):
            return cpath
        import shutil
        neff = orig(ant_bir_str, compile_dir_path, neff_name=neff_name, **kw)
        shutil.copy(neff, cpath)
        return cpath

    bass2jax.compile_bir_kernel = cached
    bass2jax._my_cache_installed = True


def run_spmd(nc, in_maps, n_cores, profile_dir=None):
    import jax
    from jax.sharding import Mesh, PartitionSpec
    from jax.experimental.shard_map import shard_map
    from concourse.bass2jax import (_bass_exec_p, partition_id_tensor,
                                    install_neuronx_cc_hook)
    _install_neff_cache()
    install_neuronx_cc_hook()

    partition_name = nc.partition_id_tensor.name if nc.partition_id_tensor else None
    in_names, out_names, out_avals, zero_outs = [], [], [], []
    for alloc in nc.m.functions[0].allocations:
        if not isinstance(alloc, mybir.MemoryLocationSet):
            continue
        name = alloc.memorylocations[0].name
        if alloc.kind == "ExternalInput":
            if name != partition_name:
                in_names.append(name)
        elif alloc.kind == "ExternalOutput":
            shape = tuple(alloc.tensor_shape)
            dtype = mybir.dt.np(alloc.dtype)
            out_names.append(name)
            out_avals.append(jax.core.ShapedArray(shape, dtype))
            zero_outs.append(np.zeros(shape, dtype))
    n_params = len(in_names)
    n_outs = len(out_avals)
    in_names.extend(out_names)
    if partition_name is not None:
        in_names.append(partition_name)
    donate = tuple(range(n_params, n_params + n_outs))

    def _body(*args):
        operands = list(args)
        if partition_name is not None:
            operands.append(partition_id_tensor())
        outs = _bass_exec_p.bind(
            *operands, out_avals=tuple(out_avals), in_names=tuple(in_names),
            out_names=tuple(out_names), lowering_input_output_aliases=(),
            sim_require_finite=True, sim_require_nnan=True, nc=nc)
        return tuple(outs)

    _body.__name__ = "u" + uuid.uuid4().hex[:12] + "_body"
    devices = jax.devices()[:n_cores]
    mesh = Mesh(np.asarray(devices), ("core",))
    sharded = jax.jit(
        shard_map(_body, mesh=mesh,
                  in_specs=(PartitionSpec("core"),) * (n_params + n_outs),
                  out_specs=(PartitionSpec("core"),) * n_outs,
                  check_rep=False),
        donate_argnums=donate, keep_unused=True)
    per_core = [[np.asarray(m[name]) for name in in_names[:n_params]]
                for m in in_maps]
    concat_in = [np.concatenate([per_core[c][i] for c in range(n_cores)], axis=0)
                 for i in range(n_params)]
    concat_zeros = [np.zeros((n_cores * z.shape[0], *z.shape[1:]), z.dtype)
                    for z in zero_outs]
    exec_ns = None
    if profile_dir is not None:
        from trn_agent_boot.trn_boot import _ntff_profile_via_ctypes
        if 'antenv.axon_hooks' not in sys.modules:
            mod = types.ModuleType('antenv.axon_hooks')
            _h = [None]
            mod.set_axon_ntff_profile_hook = lambda h: _h.__setitem__(0, h)
            mod.get_axon_ntff_profile_hook = lambda: _h[0]
            sys.modules['antenv.axon_hooks'] = mod
            import antenv
            antenv.axon_hooks = mod
        import antenv.axon_hooks as ah
        if ah.get_axon_ntff_profile_hook() is None:
            ah.set_axon_ntff_profile_hook(
                _ntff_profile_via_ctypes('/opt/axon/libaxon_pjrt.so'))
        hook = ah.get_axon_ntff_profile_hook()
        os.makedirs(profile_dir, exist_ok=True)
        compiled = sharded.lower(*concat_in, *concat_zeros).compile()
        with hook(profile_dir, [0]):
            out_arrs = compiled(*concat_in, *concat_zeros)
            out_arrs = [np.asarray(a) for a in out_arrs]
        exec_ns = _exec_time_from_ntff(profile_dir, nc)
    else:
        out_arrs = sharded(*concat_in, *concat_zeros)
        out_arrs = [np.asarray(a) for a in out_arrs]
    results = [
        {name: out_arrs[i].reshape(n_cores, *out_avals[i].shape)[c]
         for i, name in enumerate(out_names)}
        for c in range(n_cores)]
    return results, exec_ns


def _exec_time_from_ntff(profile_dir, nc):
    try:
        import gauge.profiler
        from concourse.bass_utils import _process_ntff_profile
        from concourse._compat import FishPath
        if not glob.glob(os.path.join(profile_dir, "*_body*.ntff")):
            return None
        profile = gauge.profiler.Profile(
            profile_path=FishPath(profile_dir), kernel_dev_mode=True,
            profile_on_exit=False, bass_kernel=nc.m, offline_processing=True,
            fname="*_body*", metadata={})
        r = _process_ntff_profile(profile, profile_dir, nc, [0], None, False,
                                  {}, False)
        return r.exec_time_ns
    except Exception:
        return None


_prog_cache = {}


def kernel(**inputs) -> np.ndarray:
    B, S_full, D = 2, 2048, 1024
    S, E = 512, 2048
    key = (D, S, E)
    if key not in _prog_cache:
        _prog_cache[key] = build_program(D=D, H=16, FF=4096, S=S, E=E,
                                         n_cores=8)
    nc = _prog_cache[key]
    in_maps = []
    for c in range(8):
        b, q = c // 4, c % 4
        in_maps.append(make_inputs_for_core(inputs, b, q * S))
    results, _ = run_spmd(nc, in_maps, 8)
    out = np.zeros((B, S_full, D), np.float32)
    for c in range(8):
        b, q = c // 4, c % 4
        out[b, q * S:(q + 1) * S] = results[c]["out_blk"]
    return out
